# revision 3
# baseline (speedup 1.0000x reference)
"""HEX loss kernel for Trainium2 (8 NeuronCores, batch-parallel).

Math: the chain junction tree potential factorizes per variable
(pot[c,i,j] = exp(s_i*a_c) * exp(s_j*b_c), and each interior fs[v] is
split fs[v]/2 over its two cliques), so the joint distribution is a
product of independent Bernoullis with P(y_v=1) = sigmoid(fs[v]).
Hence pMargin[labels[b], b] = sigmoid(fs[b, labels[b]]) and

    loss = mean_b softplus(-fs[b, labels[b]])

(verified to 1.4e-16 rel err against the f64 reference).

Device work per core (B_loc = 4096 rows): stream fs (4 MB), select
fs[b, labels[b]] via an iota==label mask + fused multiply-reduce.
The gen3 ACT tables have no Ln/Softplus, so softplus(-x) is computed
as ln(z), z = 1 + exp(-x), via root-doubling: w = z^(1/256) with 8
chained Sqrts (first one fuses the +1 as bias), then
ln z = 256 * ln(w) with a 4-term alternating series in s = w - 1.
Per-group partial sums [128, n_groups] come back; host sums / B.
"""

import numpy as np

B = 32768
V = 256
N_CORES = 8
BL = B // N_CORES          # 4096 rows per core
P = 128                    # SBUF partitions
RPP = 8                    # rows per partition per DMA group
GROUP_ROWS = P * RPP       # 1024 rows per group (1 MB)
N_GROUPS = BL // GROUP_ROWS  # 4
NT = BL // P               # 32 selected values per partition
N_SQRT = 8
LOG_SCALE = float(1 << N_SQRT)  # 256

_CACHE = {}


def _build():
    from contextlib import ExitStack

    import concourse.bass as bass
    import concourse.tile as tile
    from concourse import bacc, mybir

    f32 = mybir.dt.float32
    Alu = mybir.AluOpType
    Act = mybir.ActivationFunctionType

    nc = bacc.Bacc(
        "TRN2",
        target_bir_lowering=False,
        debug=False,
        enable_asserts=True,
        num_devices=N_CORES,
    )

    fs_d = nc.dram_tensor("fs", [BL, V], f32, kind="ExternalInput").ap()
    lab_d = nc.dram_tensor("labt", [P, NT], f32, kind="ExternalInput").ap()
    out_d = nc.dram_tensor("out", [P, N_GROUPS], f32, kind="ExternalOutput").ap()

    with tile.TileContext(nc) as tc, ExitStack() as ctx:
        const_pool = ctx.enter_context(tc.tile_pool(name="const", bufs=1))
        fs_pool = ctx.enter_context(tc.tile_pool(name="fs", bufs=3))
        mask_pool = ctx.enter_context(tc.tile_pool(name="mask", bufs=4))
        scratch_pool = ctx.enter_context(tc.tile_pool(name="scratch", bufs=2))
        sel_pool = ctx.enter_context(tc.tile_pool(name="sel", bufs=2))
        ep_pool = ctx.enter_context(tc.tile_pool(name="ep", bufs=2))

        iota = const_pool.tile([P, V], f32)
        nc.gpsimd.iota(
            iota[:],
            pattern=[[1, V]],
            base=0,
            channel_multiplier=0,
            allow_small_or_imprecise_dtypes=True,
        )

        labt = const_pool.tile([P, NT], f32)
        nc.sync.dma_start(out=labt[:], in_=lab_d[:])

        part = const_pool.tile([P, N_GROUPS], f32, tag="part")

        # fs rows g*1024 .. (g+1)*1024, partition p <- 8 consecutive rows
        fs_view = fs_d.rearrange("(g p j) v -> g p (j v)", g=N_GROUPS, p=P, j=RPP)

        for g in range(N_GROUPS):
            fst = fs_pool.tile([P, RPP * V], f32)
            nc.sync.dma_start(out=fst[:], in_=fs_view[g])
            sel = sel_pool.tile([P, RPP], f32)
            for j in range(RPP):
                t = g * RPP + j
                mask = mask_pool.tile([P, V], f32)
                nc.gpsimd.tensor_scalar(
                    mask[:],
                    iota[:],
                    labt[:, t : t + 1],
                    None,
                    Alu.is_equal,
                )
                scratch = scratch_pool.tile([P, V], f32)
                nc.vector.tensor_mul(scratch[:], fst[:, j * V : (j + 1) * V], mask[:])
                junk = scratch_pool.tile([P, V], f32, tag="junk")
                nc.scalar.activation(
                    junk[:],
                    scratch[:],
                    Act.Copy,
                    accum_out=sel[:, j : j + 1],
                )

            # epilogue for this group: partial[p] = sum_j softplus(-sel[p, j])
            # u = exp(-sel); w = (1 + u)^(1/256) via 8 sqrts; s = w - 1;
            # softplus = 256 * (s - s^2/2 + s^3/3 - s^4/4)
            u = ep_pool.tile([P, RPP], f32, tag="u")
            nc.scalar.activation(u[:], sel[:], Act.Exp, scale=-1.0)
            w = ep_pool.tile([P, RPP], f32, tag="w")
            nc.scalar.activation(w[:], u[:], Act.Sqrt, bias=1.0)
            for _ in range(N_SQRT - 1):
                w2 = ep_pool.tile([P, RPP], f32, tag="w")
                nc.scalar.activation(w2[:], w[:], Act.Sqrt)
                w = w2
            s = ep_pool.tile([P, RPP], f32, tag="s")
            nc.vector.tensor_scalar(s[:], w[:], -1.0, None, Alu.add)
            # Horner: e = 1 - s*(1/2 - s*(1/3 - s/4)); softplus = 256*s*e
            c = ep_pool.tile([P, RPP], f32, tag="c")
            nc.vector.tensor_scalar(c[:], s[:], -0.25, 1.0 / 3.0, Alu.mult, Alu.add)
            sc = ep_pool.tile([P, RPP], f32, tag="sc")
            nc.vector.tensor_mul(sc[:], s[:], c[:])
            d = ep_pool.tile([P, RPP], f32, tag="d")
            nc.vector.tensor_scalar(d[:], sc[:], -1.0, 0.5, Alu.mult, Alu.add)
            sd = ep_pool.tile([P, RPP], f32, tag="sd")
            nc.vector.tensor_mul(sd[:], s[:], d[:])
            e = ep_pool.tile([P, RPP], f32, tag="e")
            nc.vector.tensor_scalar(e[:], sd[:], -1.0, 1.0, Alu.mult, Alu.add)
            se = ep_pool.tile([P, RPP], f32, tag="se")
            nc.vector.tensor_mul(se[:], s[:], e[:])
            jep = ep_pool.tile([P, RPP], f32, tag="jep")
            nc.scalar.activation(
                jep[:],
                se[:],
                Act.Copy,
                scale=LOG_SCALE,
                accum_out=part[:, g : g + 1],
            )

        nc.sync.dma_start(out=out_d[:], in_=part[:])

    nc.compile()
    return nc


def _get_nc():
    if "nc" not in _CACHE:
        _CACHE["nc"] = _build()
    return _CACHE["nc"]


def _shard_inputs(fs, labels):
    fs = np.ascontiguousarray(np.asarray(fs, dtype=np.float32))
    labels = np.asarray(labels)
    in_maps = []
    for c in range(N_CORES):
        fs_loc = fs[c * BL : (c + 1) * BL]
        lab_loc = labels[c * BL : (c + 1) * BL]
        # labt[p, g*RPP + j] = lab_loc[g*GROUP_ROWS + p*RPP + j]
        labt = (
            lab_loc.reshape(N_GROUPS, P, RPP)
            .transpose(1, 0, 2)
            .reshape(P, NT)
            .astype(np.float32)
        )
        in_maps.append({"fs": fs_loc, "labt": np.ascontiguousarray(labt)})
    return in_maps


def kernel(fs, labels, _trace=False, _trace_kwargs=None):
    from concourse.bass_utils import run_bass_kernel_spmd

    nc = _get_nc()
    in_maps = _shard_inputs(fs, labels)
    res = run_bass_kernel_spmd(
        nc,
        in_maps,
        core_ids=list(range(N_CORES)),
        trace=_trace,
        **(_trace_kwargs or {}),
    )
    total = np.float64(0.0)
    for c in range(N_CORES):
        total += res.results[c]["out"].astype(np.float64).sum()
    loss = total / np.float64(B)
    if _trace:
        return np.float64(loss), res
    return np.asarray(loss, dtype=np.float64)
